# revision 11
# baseline (speedup 1.0000x reference)
"""CoAttention kernel for Trainium2, data-parallel over batch across 8 NeuronCores.

Per core (one batch element b):
    query = data1[b] @ Wq + bq                      # [2048, 256]
    key   = data2[b] @ Wk + bk                      # [2048, 256]
    attn  = softmax(SCALE * query @ key^T)
    out   = attn @ key + query

v2 strategy (vs the 112us baseline):
  - K-side projection runs fp8 DoubleRow (d2 cast to fp8, transposed on PE,
    Wk fp8-DR stationary) producing kt [d,k]; key2 [k,d|1] is built by PE
    transposes OF kt (fp8) instead of a second full projection.
  - scores PSUM is bf16 (half the banks); exp reads it directly.
  - d1 transposes: u0-u3 on PE (psum->sbuf copies split ACT-front/DVE),
    u4-u7 via the DMA crossbar transpose (InstDmaTransposeAnt) - zero
    engine time, runs on the otherwise-idle mid-kernel DMA pool.
  - The Q residual [q,d] (q_sb) is produced by DMA-transposing qtbf; the
    residual add runs on GPSIMD (sbuf-only), normalize mult on DVE.
  - Weights load as plain HWDGE fp32 + DVE casts (no slow SWDGE).
  - DMA issue order hand-tuned so the k-side chain (d2 chunk0 + Wk) and the
    q-side chain (d1 u0 + Wq) both start ASAP; d1 streams in 8 chunks.
  - Emission order keeps the PE continuously busy (trn2 ramps the PE clock
    to 2.4GHz only after ~3us of uninterrupted execution); ctx chains are
    emitted stepwise, interleaved with scores/QT fillers that match the
    serial exp pace on ACT.
  - Output is written bf16 (host upcasts) halving output DMA.
"""

import sys

if "/opt/trn_rl_repo" not in sys.path:
    sys.path.insert(0, "/opt/trn_rl_repo")

from contextlib import ExitStack

import numpy as np

import concourse.bass as bass  # noqa: F401
import concourse.mybir as mybir
import concourse.tile as tile
from concourse import bacc
from concourse.bass_utils import run_bass_kernel_spmd
from concourse.masks import make_identity

B, LQ, LK, DIN, D = 8, 2048, 2048, 1024, 256
N_CORES = 8
SCALE = float(1.0 / np.sqrt(1024.0).astype(np.float32))

BF16 = mybir.dt.bfloat16
FP8 = mybir.dt.float8e4
F32 = mybir.dt.float32
AF = mybir.ActivationFunctionType
PM_DR = mybir.MatmulPerfMode.DoubleRow
ADD = mybir.AluOpType.add
MULT = mybir.AluOpType.mult

NU = 8          # d1 processed in 8 chunks of 256 q rows
N_PE_U = 4      # d1T chunks transposed on PE (rest via DMA xbar transpose)
KB = LK // 128  # 16 k blocks
QB = LQ // 128  # 16 q blocks


def _build():
    nc = bacc.Bacc("TRN2", target_bir_lowering=False, debug=False)
    d1 = nc.dram_tensor("data1", [LQ, DIN], F32, kind="ExternalInput").ap()
    d2 = nc.dram_tensor("data2", [LK, D], F32, kind="ExternalInput").ap()
    wq = nc.dram_tensor("Wq", [DIN, D], F32, kind="ExternalInput").ap()
    wk = nc.dram_tensor("Wk", [D, D], F32, kind="ExternalInput").ap()
    bq = nc.dram_tensor("bq", [D], F32, kind="ExternalInput").ap()
    bk = nc.dram_tensor("bk", [D], F32, kind="ExternalInput").ap()
    out = nc.dram_tensor("out", [LQ, D], BF16, kind="ExternalOutput").ap()

    with tile.TileContext(nc) as tc, ExitStack() as ctx:
        const = ctx.enter_context(tc.tile_pool(name="const", bufs=1))
        big = ctx.enter_context(tc.tile_pool(name="big", bufs=1))
        st1 = ctx.enter_context(tc.tile_pool(name="st1", bufs=2))
        st1b = ctx.enter_context(tc.tile_pool(name="st1b", bufs=3))
        outp = ctx.enter_context(tc.tile_pool(name="outp", bufs=8))
        small = ctx.enter_context(tc.tile_pool(name="small", bufs=4))
        ps_sc = ctx.enter_context(tc.tile_pool(name="ps_sc", bufs=2, space="PSUM"))
        ps_cx = ctx.enter_context(tc.tile_pool(name="ps_cx", bufs=1, space="PSUM"))
        front_ps = ExitStack()
        ps_a = front_ps.enter_context(tc.tile_pool(name="ps_a", bufs=2, space="PSUM"))
        ps_t8 = front_ps.enter_context(
            tc.tile_pool(name="ps_t8", bufs=1, space="PSUM"))

        # ---------------- constants ----------------
        ident_bf = const.tile([128, 128], BF16, tag="ident_bf")
        make_identity(nc, ident_bf[:])
        ident_f8 = const.tile([128, 128], FP8, tag="ident_f8")
        make_identity(nc, ident_f8[:])
        bq_col = const.tile([128, 2], F32, tag="bq_col")
        bk_col = const.tile([128, 2], F32, tag="bk_col")

        # ---------------- persistent sbuf tensors ----------------
        d2_st = big.tile([128, 16, D], F32, tag="d2_st")
        d2f8 = big.tile([128, 16, D], FP8, tag="d2f8")
        d2T = big.tile([128, 2, LK], FP8, tag="d2T")       # [i%128, i//128, k]
        wk_st = big.tile([128, 2, D], F32, tag="wk_st")
        wk_f8 = big.tile([128, 2, D], FP8, tag="wk_f8")    # [i%128, i//128, d]
        wq_st = big.tile([128, 8, D], F32, tag="wq_st")
        wq_sb = big.tile([128, 8, D], BF16, tag="wq_sb")   # [i%128, i//128, d]
        kt_sb = big.tile([128, 2, LK], FP8, tag="kt_sb")   # [d%128, d//128, k]
        key2 = big.tile([128, KB // 2, 2, 260], FP8, tag="key2")  # [k%128, kp, s, d|1]
        d1T = big.tile([128, 8, LQ], BF16, tag="d1T")      # [i%128, i//128, q]
        qt_sb = big.tile([128, 2, LQ], FP8, tag="qt_sb")   # [d%128, d//128, q]
        qtbf = big.tile([128, 2, LQ], BF16, tag="qtbf")
        q_sb = big.tile([128, QB, D], BF16, tag="q_sb")    # residual Q [q%128, qb, d]
        expT = [
            big.tile([128, KB // 2, 2, LQ // 2], FP8, tag=f"expT{h}", name=f"expT{h}")
            for h in range(2)
        ]

        # ones column for the softmax denominator (written once, before the
        # psum copies that fill cols 0:256)
        nc.gpsimd.memset(key2[:, :, :, 256:260], 1.0)

        # ---------------- DMA issue order (sync queue, hand-tuned) ----------------
        # k-chain needs d2 chunk0 + wk first; q-chain needs wq + d1 u0/u1.
        def d2_load(c):  # c in 0..3, 512 k rows each
            nc.sync.dma_start(
                out=d2_st[:, c * 4:(c + 1) * 4, :],
                in_=d2[c * 512:(c + 1) * 512, :].rearrange("(t p) i -> p t i", p=128),
            )

        d1_st = [st1.tile([128, 2, DIN], F32, tag="d1st", name=f"d1st{u}")
                 for u in range(NU)]
        d1bf = [st1b.tile([128, 2, DIN], BF16, tag="d1bf", name=f"d1bf{u}")
                for u in range(NU)]

        def d1_load(u):  # 256 q rows
            nc.sync.dma_start(
                out=d1_st[u][:],
                in_=d1[u * 256:(u + 1) * 256, :].rearrange("(t p) i -> p t i", p=128),
            )

        d1_load(0)
        d1_load(1)
        for hh in range(2):
            nc.sync.dma_start(
                out=wq_st[:, hh * 4:(hh + 1) * 4, :],
                in_=wq[hh * 512:(hh + 1) * 512, :].rearrange(
                    "(c p) d -> p c d", p=128),
            )
        d1_load(2)
        d1_load(3)
        d2_load(0)
        nc.sync.dma_start(out=wk_st[:], in_=wk.rearrange("(s p) d -> p s d", p=128))
        nc.sync.dma_start(out=bk_col[:], in_=bk.rearrange("(c p) -> p c", p=128))
        nc.sync.dma_start(out=bq_col[:], in_=bq.rearrange("(c p) -> p c", p=128))
        d2_load(1)
        d1_load(4)
        d2_load(2)
        d2_load(3)
        for u in range(5, NU):
            d1_load(u)

        # ---------------- unit definitions ----------------
        def wk_cast():
            nc.vector.tensor_copy(wk_f8[:], wk_st[:])

        def wq_cast():
            nc.vector.tensor_copy(wq_sb[:], wq_st[:])

        def d2_cast(c):
            nc.vector.tensor_copy(d2f8[:, c * 4:(c + 1) * 4, :],
                                  d2_st[:, c * 4:(c + 1) * 4, :])

        def d2t_unit(c):
            # transpose 4 k-tiles (512 k rows) x 2 i-chunks -> d2T fp8.
            # fp8 transpose-mode psum writes require element step 2.
            pt = ps_t8.tile([128, 2048], FP8, tag="ps_t8", name=f"ptd2_{c}")
            ptv = pt[:].rearrange("p (n two) -> p two n", two=2)[:, 0, :]
            for ic in range(2):
                for j in range(4):
                    kt = c * 4 + j
                    nc.tensor.transpose(
                        ptv[:, ic * 512 + j * 128: ic * 512 + (j + 1) * 128],
                        d2f8[:, kt, ic * 128:(ic + 1) * 128],
                        ident_f8[:],
                    )
            nc.vector.tensor_copy(
                d2T[:, :, c * 512:(c + 1) * 512],
                pt[:].rearrange("p (s k two) -> p two s k", two=2, k=512)[:, 0],
            )

        def kt_unit(nk):
            # kt[d, k-chunk] via fp8 DoubleRow; bias added on the psum->fp8 cast
            for dc in range(2):
                ps = ps_a.tile([128, 512], F32, tag="ps_a", name=f"kt{nk}_{dc}")
                nc.tensor.matmul(
                    ps[:],
                    lhsT=wk_f8[:, :, dc * 128:(dc + 1) * 128],
                    rhs=d2T[:, :, nk * 512:(nk + 1) * 512],
                    perf_mode=PM_DR,
                    start=True,
                    stop=True,
                )
                nc.vector.tensor_scalar(
                    kt_sb[:, dc, nk * 512:(nk + 1) * 512], ps[:],
                    bk_col[:, dc:dc + 1], None, ADD,
                )

        def key2_unit(g):
            # transpose kt fp8 -> key2 [k, d] for kb in [4g, 4g+4)
            pt = ps_t8.tile([128, 2048], FP8, tag="ps_t8", name=f"ptk2_{g}")
            ptv = pt[:].rearrange("p (n two) -> p two n", two=2)[:, 0, :]
            for j in range(4):
                kb = g * 4 + j
                for dc in range(2):
                    nc.tensor.transpose(
                        ptv[:, j * 256 + dc * 128: j * 256 + (dc + 1) * 128],
                        kt_sb[:, dc, kb * 128:(kb + 1) * 128],
                        ident_f8[:],
                    )
            nc.vector.tensor_copy(
                key2[:, g * 2:(g + 1) * 2, :, 0:256],
                pt[:].rearrange("p (a s d two) -> p two a s d", two=2, s=2, d=256)[:, 0],
            )

        def d1_cast(u):
            nc.vector.tensor_copy(d1bf[u][:], d1_st[u][:])

        def d1_cast_pool(u):
            nc.gpsimd.tensor_copy(d1bf[u][:], d1_st[u][:])

        def d1t_dma_unit(u, eng="sync"):
            e = nc.scalar if eng == "act" else nc.sync
            for j in range(2):
                e.dma_start_transpose(
                    out=d1T[:, :, (u * 2 + j) * 128:(u * 2 + j + 1) * 128],
                    in_=d1bf[u][:, j, :],
                )

        def qt_unit(u, qtbf_engine):
            # QT[d, q-chunk] = Wq^T @ d1T + bq, bf16 proj with fp32 psum.
            # dc0/dc1 are sequential accumulation groups in one psum bank.
            ps = ps_a.tile([128, 512], F32, tag="ps_a", name=f"qt{u}")
            for dc in range(2):
                p = ps[:, dc * 256:(dc + 1) * 256]
                for ic in range(8):
                    nc.tensor.matmul(
                        p,
                        lhsT=wq_sb[:, ic, dc * 128:(dc + 1) * 128],
                        rhs=d1T[:, ic, u * 256:(u + 1) * 256],
                        start=(ic == 0),
                        stop=(ic == 7),
                    )
                nc.vector.tensor_scalar(
                    qt_sb[:, dc, u * 256:(u + 1) * 256], p,
                    bq_col[:, dc:dc + 1], None, ADD,
                )
                if qtbf_engine == "act":
                    nc.scalar.activation(
                        qtbf[:, dc, u * 256:(u + 1) * 256], p, AF.Identity,
                        bias=bq_col[:, dc:dc + 1],
                    )
                else:
                    nc.vector.tensor_scalar(
                        qtbf[:, dc, u * 256:(u + 1) * 256], p,
                        bq_col[:, dc:dc + 1], None, ADD,
                    )

        def qres_dma(h, dc):
            # q_sb[q, qb, d] = Q via DMA xbar transpose of qtbf
            nc.sync.dma_start_transpose(
                out=q_sb[:, h * 8:(h + 1) * 8, dc * 128:(dc + 1) * 128],
                in_=qtbf[:, dc, h * 1024:(h + 1) * 1024],
            )

        def scores_unit(km, h):
            ps = ps_sc.tile([128, 1024], F32, tag="ps_sc", name=f"sc{h}_{km}")
            for half in range(2):
                nq = h * 2 + half
                nc.tensor.matmul(
                    ps[:, half * 512:(half + 1) * 512],
                    lhsT=kt_sb[:, :, km * 128:(km + 1) * 128],
                    rhs=qt_sb[:, :, nq * 512:(nq + 1) * 512],
                    perf_mode=PM_DR,
                    start=True,
                    stop=True,
                )
            nc.scalar.activation(
                expT[h][:, km // 2, km % 2, :], ps[:], AF.Exp, scale=SCALE,
            )

        out_sb = [outp.tile([128, 2, D], BF16, tag="out_sb", name=f"osb{qg}")
                  for qg in range(8)]

        def ctx_steps(qb, pool=None, tag="ps_cx"):
            # generator of the 8 chained DR matmuls + the normalize epilogue
            h, qq = qb // 8, qb % 8
            pool = pool if pool is not None else ps_cx
            pc_full = pool.tile([128, 512], F32, tag=tag, name=f"cx{qb}")
            pc = pc_full[:, 0:257]
            for kp in range(KB // 2):
                def step(kp=kp, pc=pc):
                    nc.tensor.matmul(
                        pc,
                        lhsT=expT[h][:, kp, :, qq * 128:(qq + 1) * 128],
                        rhs=key2[:, kp, :, 0:257],
                        perf_mode=PM_DR,
                        start=(kp == 0),
                        stop=(kp == KB // 2 - 1),
                    )
                yield step

            def epilogue(pc=pc, qb=qb):
                qg, sl = qb // 2, qb % 2
                rc = small.tile([128, 1], F32, tag="recip")
                nc.vector.reciprocal(rc[:], pc[:, 256:257])
                osl = out_sb[qg][:, sl, :]
                nc.vector.tensor_scalar(osl, pc[:, 0:256], rc[:], None, MULT)
                nc.gpsimd.tensor_tensor(osl, osl, q_sb[:, qb, :], ADD)
                if sl == 1:
                    nc.sync.dma_start(
                        out=out[qg * 256:(qg + 1) * 256, :].rearrange(
                            "(t p) d -> p t d", p=128),
                        in_=out_sb[qg][:],
                    )
            yield epilogue

        # ---------------- emission: front phase ----------------
        d1_cast(0)
        d1t_dma_unit(0, "act")
        d1_cast(1)
        d1t_dma_unit(1, "act")
        wq_cast()
        qt_unit(0, "act")
        d1_cast(2)
        d1t_dma_unit(2, "act")
        qt_unit(1, "act")
        d1_cast(3)
        d1t_dma_unit(3, "act")
        d2_cast(0)
        wk_cast()
        d2t_unit(0)
        kt_unit(0)
        key2_unit(0)
        qt_unit(2, "vec")
        qt_unit(3, "vec")

        # ---------------- back half: scores h0 + fillers ----------------
        # fillers are consumed between scores/ctx units in emission order;
        # each is a 0-arg callable
        fillers = []
        fillers.append(lambda: d2_cast(1))
        fillers.append(lambda: d2t_unit(1))
        fillers.append(lambda: kt_unit(1))
        fillers.append(lambda: key2_unit(1))
        fillers.append(lambda: d2_cast(2))
        fillers.append(lambda: d2t_unit(2))
        fillers.append(lambda: kt_unit(2))
        fillers.append(lambda: key2_unit(2))
        fillers.append(lambda: d2_cast(3))
        fillers.append(lambda: d2t_unit(3))
        fillers.append(lambda: kt_unit(3))
        fillers.append(lambda: key2_unit(3))
        fillers.append(lambda: d1_cast_pool(4))
        fillers.append(lambda: d1t_dma_unit(4))
        fillers.append(lambda: qt_unit(4, "vec"))
        fillers.append(lambda: d1_cast_pool(5))
        fillers.append(lambda: d1t_dma_unit(5))
        fillers.append(lambda: qt_unit(5, "vec"))
        fillers.append(lambda: qres_dma(0, 0))
        fillers.append(lambda: qres_dma(0, 1))
        fillers.append(lambda: d1_cast(6))
        fillers.append(lambda: d1t_dma_unit(6))
        fillers.append(lambda: qt_unit(6, "vec"))
        fillers.append(lambda: d1_cast(7))
        fillers.append(lambda: d1t_dma_unit(7))
        fillers.append(lambda: qt_unit(7, "vec"))
        fillers.append(lambda: qres_dma(1, 0))
        fillers.append(lambda: qres_dma(1, 1))

        fi = [0]

        def fill(n):
            while n > 0 and fi[0] < len(fillers):
                fillers[fi[0]]()
                fi[0] += 1
                n -= 1

        # scores h0: all fillers are consumed here (their PE weight roughly
        # matches the serial exp pace on ACT)
        for km in range(KB):
            scores_unit(km, 0)
            fill(2)
        while fi[0] < len(fillers):
            fillers[fi[0]]()
            fi[0] += 1

        # front psum pools (QT/kt/transposes) are done - release their 3
        # banks so the ctx-h1 chains can hold 4 psum slots at once
        front_ps.close()
        ps_cx2 = ctx.enter_context(
            tc.tile_pool(name="ps_cx2", bufs=3, space="PSUM"))

        # ctx h0 (single slot, chases exp-h0) interleaved with scores h1
        sc1 = [lambda km=km: scores_unit(km, 1) for km in range(KB)]
        si = [0]

        def fill_sc1(n):
            while n > 0 and si[0] < len(sc1):
                sc1[si[0]]()
                si[0] += 1
                n -= 1

        for qb in range(8):
            for step in ctx_steps(qb):
                step()
                fill_sc1(1)
        while si[0] < len(sc1):
            sc1[si[0]]()
            si[0] += 1

        # ctx h1: 4 chains resident (1x ps_cx + 3x ps_cx2) so most chain
        # steps complete while exps are still streaming; only the last
        # 4 chains run after the final exp.
        h1_specs = [(qb, (ps_cx2, "ps_cx2")) for qb in (8, 9, 10)] + \
                   [(11, (ps_cx, "ps_cx"))] + \
                   [(qb, (ps_cx2, "ps_cx2")) for qb in (12, 13, 14)] + \
                   [(15, (ps_cx, "ps_cx"))]
        gens = []
        for qb, (pool, tag) in h1_specs[:4]:
            gens.append(ctx_steps(qb, pool, tag))
        # round-robin the first 4 chains stepwise (they gate on exps)
        alive = list(gens)
        while alive:
            nxt = []
            for g in alive:
                try:
                    next(g)()
                    nxt.append(g)
                except StopIteration:
                    pass
            alive = nxt
        for qb, (pool, tag) in h1_specs[4:]:
            for step in ctx_steps(qb, pool, tag):
                step()

    nc.compile()
    return nc


_NC = None


def _get_nc():
    global _NC
    if _NC is None:
        _NC = _build()
    return _NC


def kernel(data1, data2, Wq, bq, Wk, bk):
    data1 = np.asarray(data1, dtype=np.float32)
    data2 = np.asarray(data2, dtype=np.float32)
    Wq = np.ascontiguousarray(np.asarray(Wq, dtype=np.float32))
    Wk = np.ascontiguousarray(np.asarray(Wk, dtype=np.float32))
    bq = np.ascontiguousarray(np.asarray(bq, dtype=np.float32))
    bk = np.ascontiguousarray(np.asarray(bk, dtype=np.float32))

    nc = _get_nc()
    in_maps = [
        {
            "data1": np.ascontiguousarray(data1[b]),
            "data2": np.ascontiguousarray(data2[b]),
            "Wq": Wq,
            "Wk": Wk,
            "bq": bq,
            "bk": bk,
        }
        for b in range(B)
    ]
    res = run_bass_kernel_spmd(nc, in_maps, core_ids=list(range(N_CORES)))
    return np.stack(
        [np.asarray(res.results[i]["out"]).astype(np.float32) for i in range(B)],
        axis=0,
    )


# revision 14
# speedup vs baseline: 1.3781x; 1.3781x over previous
"""CoAttention kernel for Trainium2, data-parallel over batch across 8 NeuronCores.

Per core (one batch element b):
    query = data1[b] @ Wq + bq                      # [2048, 256]
    key   = data2[b] @ Wk + bk                      # [2048, 256]
    attn  = softmax(SCALE * query @ key^T)
    out   = attn @ key + query

v3 strategy (vs the 112us baseline):
  - Activation transposes run as fp32 PE transposes straight from the
    staged fp32 data (2 cyc/col but no separate dtype-cast pass); the
    psum->sbuf copy does the downcast (d1T -> bf16, d2T -> fp8).
    Exception: d1 chunks u4-u7 (latency-tolerant) go bf16-cast + DMA
    crossbar transpose on the otherwise-idle mid-kernel DMA pool.
  - K projection runs fp8 DoubleRow twice: kt [d,k] (scores stationary,
    bias via per-partition add) and key2 [k,d|1] directly (bias and the
    softmax-denominator ones column via a 1-row matmul of [bk|1]).
  - Inputs split across two DMA queues: d1 streams on the sync queue,
    weights + d2 on the scalar queue, so neither convoy blocks the other.
  - Emission order keeps the PE continuously busy (trn2 ramps the PE clock
    up only after ~3us of uninterrupted execution); ctx chains emit
    stepwise chasing the serial exp pace on ACT; the last 8 chains hold 4
    psum slots (the front psum pool is released mid-kernel to make room).
  - Output is written bf16 (host upcasts), halving output DMA.
"""

import sys

if "/opt/trn_rl_repo" not in sys.path:
    sys.path.insert(0, "/opt/trn_rl_repo")

from contextlib import ExitStack

import numpy as np

import concourse.bass as bass  # noqa: F401
import concourse.mybir as mybir
import concourse.tile as tile
from concourse import bacc
from concourse.bass_utils import run_bass_kernel_spmd
from concourse.masks import make_identity

B, LQ, LK, DIN, D = 8, 2048, 2048, 1024, 256
N_CORES = 8
SCALE = float(1.0 / np.sqrt(1024.0).astype(np.float32))

BF16 = mybir.dt.bfloat16
FP8 = mybir.dt.float8e4
F32 = mybir.dt.float32
AF = mybir.ActivationFunctionType
PM_DR = mybir.MatmulPerfMode.DoubleRow
ADD = mybir.AluOpType.add
MULT = mybir.AluOpType.mult

NU = 8          # d1 processed in 8 chunks of 256 q rows
KB = LK // 128  # 16 k blocks
QB = LQ // 128  # 16 q blocks


def _build():
    nc = bacc.Bacc("TRN2", target_bir_lowering=False, debug=False)
    d1 = nc.dram_tensor("data1", [LQ, DIN], F32, kind="ExternalInput").ap()
    d2 = nc.dram_tensor("data2", [LK, D], F32, kind="ExternalInput").ap()
    wq = nc.dram_tensor("Wq", [DIN, D], F32, kind="ExternalInput").ap()
    wk = nc.dram_tensor("Wk", [D, D], F32, kind="ExternalInput").ap()
    bqv = nc.dram_tensor("bq", [D], F32, kind="ExternalInput").ap()
    bkv = nc.dram_tensor("bk", [D], F32, kind="ExternalInput").ap()
    out = nc.dram_tensor("out", [LQ, D], BF16, kind="ExternalOutput").ap()

    with tile.TileContext(nc) as tc, ExitStack() as ctx:
        const = ctx.enter_context(tc.tile_pool(name="const", bufs=1))
        big = ctx.enter_context(tc.tile_pool(name="big", bufs=1))
        st1 = ctx.enter_context(tc.tile_pool(name="st1", bufs=4))
        st1b = ctx.enter_context(tc.tile_pool(name="st1b", bufs=2))
        outp = ctx.enter_context(tc.tile_pool(name="outp", bufs=8))
        small = ctx.enter_context(tc.tile_pool(name="small", bufs=4))
        ps_sc = ctx.enter_context(tc.tile_pool(name="ps_sc", bufs=2, space="PSUM"))
        ps_cx = ctx.enter_context(tc.tile_pool(name="ps_cx", bufs=1, space="PSUM"))
        front_ps = ExitStack()
        ps_a = front_ps.enter_context(tc.tile_pool(name="ps_a", bufs=2, space="PSUM"))
        ps_t = front_ps.enter_context(tc.tile_pool(name="ps_t", bufs=1, space="PSUM"))

        # ---------------- constants ----------------
        ident = const.tile([128, 128], BF16, tag="ident")
        make_identity(nc, ident[:])
        ones_row = const.tile([1, 128], BF16, tag="ones_row")
        nc.vector.memset(ones_row[:], 1.0)
        bq_col = const.tile([128, 2], F32, tag="bq_col")
        bk_col = const.tile([128, 2], F32, tag="bk_col")
        bk_row = const.tile([1, 256], F32, tag="bk_row")
        bke_row = const.tile([1, 260], BF16, tag="bke_row")

        # ---------------- persistent sbuf tensors ----------------
        d2_st = big.tile([128, 16, D], F32, tag="d2_st")
        d2bf = big.tile([128, 16, D], BF16, tag="d2bf")
        d2T = big.tile([128, 2, LK], FP8, tag="d2T")       # [i%128, i//128, k]
        wk_st = big.tile([128, 2, D], F32, tag="wk_st")
        wk_f8 = big.tile([128, 2, D], FP8, tag="wk_f8")    # [i%128, i//128, d]
        wk_f8e = big.tile([128, 2, 260], FP8, tag="wk_f8e")  # [.., d|pad0]
        wq_st = big.tile([128, 8, D], F32, tag="wq_st")
        wq_sb = big.tile([128, 8, D], BF16, tag="wq_sb")   # [i%128, i//128, d]
        kt_sb = big.tile([128, 2, LK], FP8, tag="kt_sb")   # [d%128, d//128, k]
        key2 = big.tile([128, KB // 2, 2, 260], FP8, tag="key2")  # [k%128, kp, s, d|1]
        d1T = big.tile([128, 8, LQ], BF16, tag="d1T")      # [i%128, i//128, q]
        qt_sb = big.tile([128, 2, LQ], FP8, tag="qt_sb")   # [d%128, d//128, q]
        qtbf = big.tile([128, 2, LQ], BF16, tag="qtbf")    # QT + bq in bf16
        q_sb = big.tile([128, QB, D], BF16, tag="q_sb")    # residual Q [q%128, qb, d]
        expT = [
            big.tile([128, KB // 2, 2, LQ // 2], FP8, tag=f"expT{h}", name=f"expT{h}")
            for h in range(2)
        ]

        # zero-pad wk_f8e cols 256:260 so the key2 matmul contributes 0 to
        # the ones column (which the [bk|1] bias row then sets to 1)
        nc.gpsimd.memset(wk_f8e[:, :, 256:260], 0.0)

        # ---------------- DMA issues ----------------
        d1_st = [st1.tile([128, 2, DIN], F32, tag="d1st", name=f"d1st{u}")
                 for u in range(NU)]
        d1bf = [st1b.tile([128, 2, DIN], BF16, tag="d1bf", name=f"d1bf{u}")
                for u in range(NU)]

        def d1_load(u):  # 256 q rows, 1 MB, sync queue
            nc.sync.dma_start(
                out=d1_st[u][:],
                in_=d1[u * 256:(u + 1) * 256, :].rearrange("(t p) i -> p t i", p=128),
            )

        def d2_load(c):  # c in 0..3, 512 k rows each, scalar queue
            nc.scalar.dma_start(
                out=d2_st[:, c * 4:(c + 1) * 4, :],
                in_=d2[c * 512:(c + 1) * 512, :].rearrange("(t p) i -> p t i", p=128),
            )

        for hh in range(2):
            nc.scalar.dma_start(
                out=wq_st[:, hh * 4:(hh + 1) * 4, :],
                in_=wq[hh * 512:(hh + 1) * 512, :].rearrange(
                    "(c p) d -> p c d", p=128),
            )
        d1_load(0)
        d2_load(0)
        nc.scalar.dma_start(out=wk_st[:], in_=wk.rearrange("(s p) d -> p s d", p=128))
        nc.scalar.dma_start(out=bk_col[:], in_=bkv.rearrange("(c p) -> p c", p=128))
        nc.scalar.dma_start(out=bq_col[:], in_=bqv.rearrange("(c p) -> p c", p=128))
        nc.scalar.dma_start(out=bk_row[:], in_=bkv.rearrange("(a d) -> a d", a=1))
        d1_load(1)
        d1_load(2)
        d1_load(3)
        for u in range(4, NU):
            d1_load(u)

        # ---------------- unit definitions ----------------
        def w_casts():
            nc.vector.tensor_copy(wq_sb[:], wq_st[:])
            nc.vector.tensor_copy(wk_f8[:], wk_st[:])
            nc.vector.tensor_copy(wk_f8e[:, :, 0:256], wk_st[:])
            # bke_row = [bk | 1]: ones everywhere, bk cast over cols 0:256
            nc.vector.memset(bke_row[:], 1.0)
            nc.vector.tensor_copy(bke_row[:, 0:256], bk_row[:])

        def d1t_tr(u, g, cp_engine):
            # g in 0..1: bf16 transposes of 8 [128x128] tiles of d1 chunk u
            # (i-chunks 4g..4g+4 for both q-subtiles); one [1024] copy
            pt = ps_t.tile([128, 1024], BF16, tag="ps_t", name=f"ptd1_{u}_{g}")
            for icc in range(4):
                ic = g * 4 + icc
                for j in range(2):
                    nc.tensor.transpose(
                        pt[:, (icc * 2 + j) * 128:(icc * 2 + j + 1) * 128],
                        d1bf[u][:, j, ic * 128:(ic + 1) * 128],
                        ident[:],
                    )
            dst = d1T[:, g * 4:(g + 1) * 4, u * 256:(u + 1) * 256]
            src = pt[:].rearrange("p (c q) -> p c q", c=4)
            if cp_engine == "act":
                nc.scalar.copy(dst, src)
            else:
                nc.vector.tensor_copy(dst, src)

        def d1t_pe(u, cp_engine):
            for g in range(2):
                d1t_tr(u, g, cp_engine)

        def d1_cast(u):
            nc.vector.tensor_copy(d1bf[u][:], d1_st[u][:])

        def d1t_dma(u):
            for j in range(2):
                nc.sync.dma_start_transpose(
                    out=d1T[:, :, (u * 2 + j) * 128:(u * 2 + j + 1) * 128],
                    in_=d1bf[u][:, j, :],
                )

        def d2_cast(c):
            nc.vector.tensor_copy(d2bf[:, c * 4:(c + 1) * 4, :],
                                  d2_st[:, c * 4:(c + 1) * 4, :])

        def d2t_unit(c, cp_engine):
            # bf16 transposes of d2 chunk c (4 k-tiles x 2 i-chunks); the
            # [1024] psum->sbuf copy casts to fp8
            pt = ps_t.tile([128, 1024], BF16, tag="ps_t", name=f"ptd2_{c}")
            for ic in range(2):
                for j in range(4):
                    kt = c * 4 + j
                    nc.tensor.transpose(
                        pt[:, ic * 512 + j * 128: ic * 512 + (j + 1) * 128],
                        d2bf[:, kt, ic * 128:(ic + 1) * 128],
                        ident[:],
                    )
            dst = d2T[:, :, c * 512:(c + 1) * 512]
            src = pt[:].rearrange("p (s k) -> p s k", k=512)
            if cp_engine == "act":
                nc.scalar.copy(dst, src)
            else:
                nc.vector.tensor_copy(dst, src)

        def kt_unit(nk):
            # kt[d, k-chunk] via fp8 DoubleRow; bias added on the psum cast
            for dc in range(2):
                ps = ps_a.tile([128, 512], F32, tag="ps_a", name=f"kt{nk}_{dc}")
                nc.tensor.matmul(
                    ps[:],
                    lhsT=wk_f8[:, :, dc * 128:(dc + 1) * 128],
                    rhs=d2T[:, :, nk * 512:(nk + 1) * 512],
                    perf_mode=PM_DR,
                    start=True,
                    stop=True,
                )
                nc.vector.tensor_scalar(
                    kt_sb[:, dc, nk * 512:(nk + 1) * 512], ps[:],
                    bk_col[:, dc:dc + 1], None, ADD,
                )

        def key2_unit(kb, cp_engine):
            # key2[k,d|1] = d2T^T @ [Wk|0] + [bk|1] via fp8 DR + 1-row matmul
            ps = ps_a.tile([128, 512], F32, tag="ps_a", name=f"k2_{kb}")
            p = ps[:, 0:257]
            nc.tensor.matmul(
                p,
                lhsT=d2T[:, :, kb * 128:(kb + 1) * 128],
                rhs=wk_f8e[:, :, 0:257],
                perf_mode=PM_DR,
                start=True,
                stop=False,
            )
            nc.tensor.matmul(
                p, lhsT=ones_row[:], rhs=bke_row[:, 0:257], start=False, stop=True,
            )
            dst = key2[:, kb // 2, kb % 2, 0:257]
            if cp_engine == "act":
                nc.scalar.copy(dst, p)
            else:
                nc.vector.tensor_copy(dst, p)

        def qt_unit(u, bias_engine):
            # QT[d, q-chunk] = Wq^T @ d1T + bq (bf16 matmul, fp32 psum);
            # dc0/dc1 are sequential accumulation groups in one psum bank
            ps = ps_a.tile([128, 512], F32, tag="ps_a", name=f"qt{u}")
            for dc in range(2):
                p = ps[:, dc * 256:(dc + 1) * 256]
                for ic in range(8):
                    nc.tensor.matmul(
                        p,
                        lhsT=wq_sb[:, ic, dc * 128:(dc + 1) * 128],
                        rhs=d1T[:, ic, u * 256:(u + 1) * 256],
                        start=(ic == 0),
                        stop=(ic == 7),
                    )
                nc.vector.tensor_scalar(
                    qt_sb[:, dc, u * 256:(u + 1) * 256], p,
                    bq_col[:, dc:dc + 1], None, ADD,
                )
                if bias_engine == "act":
                    nc.scalar.activation(
                        qtbf[:, dc, u * 256:(u + 1) * 256], p, AF.Identity,
                        bias=bq_col[:, dc:dc + 1],
                    )
                else:
                    nc.vector.tensor_scalar(
                        qtbf[:, dc, u * 256:(u + 1) * 256], p,
                        bq_col[:, dc:dc + 1], None, ADD,
                    )

        def qres_unit(h, qg2, cp_engine):
            # bf16 transposes of qtbf -> residual Q rows: 4 q-blocks x 2 dc
            # in one [1024] psum tile + one copy
            qb0 = h * 8 + qg2 * 4
            pt = ps_t.tile([128, 1024], BF16, tag="ps_t", name=f"qr_{h}_{qg2}")
            for b4 in range(4):
                for dc in range(2):
                    nc.tensor.transpose(
                        pt[:, (b4 * 2 + dc) * 128:(b4 * 2 + dc + 1) * 128],
                        qtbf[:, dc, (qb0 + b4) * 128:(qb0 + b4 + 1) * 128],
                        ident[:],
                    )
            dst = q_sb[:, qb0:qb0 + 4, :]
            src = pt[:].rearrange("p (b d) -> p b d", b=4)
            if cp_engine == "act":
                nc.scalar.copy(dst, src)
            else:
                nc.vector.tensor_copy(dst, src)

        def scores_unit(km, h):
            ps = ps_sc.tile([128, 1024], F32, tag="ps_sc", name=f"sc{h}_{km}")
            for half in range(2):
                nq = h * 2 + half
                nc.tensor.matmul(
                    ps[:, half * 512:(half + 1) * 512],
                    lhsT=kt_sb[:, :, km * 128:(km + 1) * 128],
                    rhs=qt_sb[:, :, nq * 512:(nq + 1) * 512],
                    perf_mode=PM_DR,
                    start=True,
                    stop=True,
                )
            nc.scalar.activation(
                expT[h][:, km // 2, km % 2, :], ps[:], AF.Exp, scale=SCALE,
            )

        out_sb = [outp.tile([128, 2, D], BF16, tag="out_sb", name=f"osb{qg}")
                  for qg in range(8)]

        def ctx_steps(qb, pool=None, tag="ps_cx"):
            # generator: 8 chained DR matmuls + the normalize epilogue
            h, qq = qb // 8, qb % 8
            pool = pool if pool is not None else ps_cx
            pc_full = pool.tile([128, 512], F32, tag=tag, name=f"cx{qb}")
            pc = pc_full[:, 0:257]
            for kp in range(KB // 2):
                def step(kp=kp, pc=pc):
                    nc.tensor.matmul(
                        pc,
                        lhsT=expT[h][:, kp, :, qq * 128:(qq + 1) * 128],
                        rhs=key2[:, kp, :, 0:257],
                        perf_mode=PM_DR,
                        start=(kp == 0),
                        stop=(kp == KB // 2 - 1),
                    )
                yield step

            def epilogue(pc=pc, qb=qb):
                qg, sl = qb // 2, qb % 2
                rc = small.tile([128, 1], F32, tag="recip")
                nc.vector.reciprocal(rc[:], pc[:, 256:257])
                osl = out_sb[qg][:, sl, :]
                nc.vector.tensor_scalar(osl, pc[:, 0:256], rc[:], None, MULT)
                nc.gpsimd.tensor_tensor(osl, osl, q_sb[:, qb, :], ADD)
                if sl == 1:
                    nc.sync.dma_start(
                        out=out[qg * 256:(qg + 1) * 256, :].rearrange(
                            "(t p) d -> p t d", p=128),
                        in_=out_sb[qg][:],
                    )
            yield epilogue

        # ---------------- emission: front phase ----------------
        w_casts()
        d1_cast(0)
        d1t_pe(0, "act")
        d1_cast(1)
        qt_unit(0, "act")
        d1t_pe(1, "act")
        d2_cast(0)
        d2t_unit(0, "vec")
        kt_unit(0)
        d1_cast(2)
        qt_unit(1, "act")
        d1t_pe(2, "vec")
        d1_cast(3)
        qt_unit(2, "vec")
        d1t_pe(3, "vec")
        qt_unit(3, "vec")
        d2_load(1)
        d2_load(2)
        d2_load(3)
        for kb in range(4):
            key2_unit(kb, "vec")

        # ---------------- back half: scores h0 + fillers ----------------
        fillers = []
        fillers.append(lambda: d2_cast(1))
        fillers.append(lambda: d2t_unit(1, "vec"))
        fillers.append(lambda: kt_unit(1))
        for kb in range(4, 8):
            fillers.append(lambda kb=kb: key2_unit(kb, "vec"))
        fillers.append(lambda: d1_cast(4))
        fillers.append(lambda: d1t_dma(4))
        fillers.append(lambda: qt_unit(4, "vec"))
        fillers.append(lambda: d2_cast(2))
        fillers.append(lambda: d2t_unit(2, "vec"))
        fillers.append(lambda: kt_unit(2))
        for kb in range(8, 12):
            fillers.append(lambda kb=kb: key2_unit(kb, "vec"))
        fillers.append(lambda: d1_cast(5))
        fillers.append(lambda: d1t_dma(5))
        fillers.append(lambda: qt_unit(5, "vec"))
        fillers.append(lambda: d2_cast(3))
        fillers.append(lambda: d2t_unit(3, "vec"))
        fillers.append(lambda: kt_unit(3))
        for kb in range(12, 16):
            fillers.append(lambda kb=kb: key2_unit(kb, "vec"))
        fillers.append(lambda: qres_unit(0, 0, "vec"))
        fillers.append(lambda: qres_unit(0, 1, "vec"))
        fillers.append(lambda: d1_cast(6))
        fillers.append(lambda: d1t_dma(6))
        fillers.append(lambda: qt_unit(6, "vec"))
        fillers.append(lambda: d1_cast(7))
        fillers.append(lambda: d1t_dma(7))
        fillers.append(lambda: qt_unit(7, "vec"))
        fillers.append(lambda: qres_unit(1, 0, "vec"))
        fillers.append(lambda: qres_unit(1, 1, "vec"))

        fi = [0]

        def fill(n):
            while n > 0 and fi[0] < len(fillers):
                fillers[fi[0]]()
                fi[0] += 1
                n -= 1

        for km in range(KB):
            scores_unit(km, 0)
            fill(2)
        while fi[0] < len(fillers):
            fillers[fi[0]]()
            fi[0] += 1

        # front psum pool done -> release 3 banks for ctx h1 multi-slot
        front_ps.close()
        ps_cx2 = ctx.enter_context(
            tc.tile_pool(name="ps_cx2", bufs=3, space="PSUM"))

        # ctx h0 (single slot, chases exp-h0) interleaved with scores h1
        sc1 = [lambda km=km: scores_unit(km, 1) for km in range(KB)]
        si = [0]

        def fill_sc1(n):
            while n > 0 and si[0] < len(sc1):
                sc1[si[0]]()
                si[0] += 1
                n -= 1

        for qb in range(8):
            for step in ctx_steps(qb):
                step()
                fill_sc1(1)
        while si[0] < len(sc1):
            sc1[si[0]]()
            si[0] += 1

        # ctx h1: 4 chains resident so most steps complete while exps
        # stream; only the last 4 chains run after the final exp
        h1_specs = [(8, (ps_cx2, "ps_cx2")), (9, (ps_cx2, "ps_cx2")),
                    (10, (ps_cx2, "ps_cx2")), (11, (ps_cx, "ps_cx")),
                    (12, (ps_cx2, "ps_cx2")), (13, (ps_cx2, "ps_cx2")),
                    (14, (ps_cx2, "ps_cx2")), (15, (ps_cx, "ps_cx"))]
        gens = [ctx_steps(qb, pool, tag) for qb, (pool, tag) in h1_specs[:4]]
        alive = list(gens)
        while alive:
            nxt = []
            for g in alive:
                try:
                    next(g)()
                    nxt.append(g)
                except StopIteration:
                    pass
            alive = nxt
        for qb, (pool, tag) in h1_specs[4:]:
            for step in ctx_steps(qb, pool, tag):
                step()

    nc.compile()
    return nc


_NC = None


def _get_nc():
    global _NC
    if _NC is None:
        _NC = _build()
    return _NC


def kernel(data1, data2, Wq, bq, Wk, bk):
    data1 = np.asarray(data1, dtype=np.float32)
    data2 = np.asarray(data2, dtype=np.float32)
    Wq = np.ascontiguousarray(np.asarray(Wq, dtype=np.float32))
    Wk = np.ascontiguousarray(np.asarray(Wk, dtype=np.float32))
    bq = np.ascontiguousarray(np.asarray(bq, dtype=np.float32))
    bk = np.ascontiguousarray(np.asarray(bk, dtype=np.float32))

    nc = _get_nc()
    in_maps = [
        {
            "data1": np.ascontiguousarray(data1[b]),
            "data2": np.ascontiguousarray(data2[b]),
            "Wq": Wq,
            "Wk": Wk,
            "bq": bq,
            "bk": bk,
        }
        for b in range(B)
    ]
    res = run_bass_kernel_spmd(nc, in_maps, core_ids=list(range(N_CORES)))
    return np.stack(
        [np.asarray(res.results[i]["out"]).astype(np.float32) for i in range(B)],
        axis=0,
    )


# revision 16
# speedup vs baseline: 1.4423x; 1.0466x over previous
"""CoAttention kernel for Trainium2, data-parallel over batch across 8 NeuronCores.

Per core (one batch element b):
    query = data1[b] @ Wq + bq                      # [2048, 256]
    key   = data2[b] @ Wk + bk                      # [2048, 256]
    attn  = softmax(SCALE * query @ key^T)          # row-constant terms cancel
    out   = attn @ key + query

Device-side strategy:
  - Activations load as fp32 (plain HWDGE DMAs, consolidated), are cast to
    bf16 on DVE, and transposed on the PE (transpose-mode, bf16 PSUM out) so
    the contraction dim lands on partitions.
  - The QT projection runs weights-stationary (bf16, fp32 PSUM accumulate,
    bias via per-partition activation bias); the residual Q [q, d] comes from
    PE transposes of the bf16 QT.
  - scoresT [k, q] orientation lets exp(scoresT) feed the context matmul
    directly as the stationary operand. The scores and context matmuls run in
    fp8e4m3 with DoubleRow packing ([128, 2, n] APs, d = slot*128 + p), which
    is safe because the attention term is tiny relative to the residual.
  - Softmax denominator is a ones-column appended to the key value matrix;
    no max-subtraction (|SCALE*scores| < ~4, exp is well-conditioned there).
  - The q range is processed in two halves, and phase units are interleaved
    in emission order (scores with next-half QT, scores with context) so PE
    matmuls overlap the serial exp work on the scalar engine.
"""

import sys

if "/opt/trn_rl_repo" not in sys.path:
    sys.path.insert(0, "/opt/trn_rl_repo")

from contextlib import ExitStack

import numpy as np

import concourse.bass as bass  # noqa: F401
import concourse.mybir as mybir
import concourse.tile as tile
from concourse import bacc
from concourse.bass_utils import run_bass_kernel_spmd
from concourse.masks import make_identity

B, LQ, LK, DIN, D = 8, 2048, 2048, 1024, 256
N_CORES = 8
SCALE = float(1.0 / np.sqrt(1024.0).astype(np.float32))

BF16 = mybir.dt.bfloat16
FP8 = mybir.dt.float8e4
F32 = mybir.dt.float32
AF = mybir.ActivationFunctionType
PM_DR = mybir.MatmulPerfMode.DoubleRow


def _build():
    nc = bacc.Bacc("TRN2", target_bir_lowering=False, debug=False)
    d1 = nc.dram_tensor("data1", [LQ, DIN], F32, kind="ExternalInput").ap()
    d2 = nc.dram_tensor("data2", [LK, D], F32, kind="ExternalInput").ap()
    wq = nc.dram_tensor("Wq", [DIN, D], F32, kind="ExternalInput").ap()
    wkx = nc.dram_tensor("Wk_ext", [D, D + 1], F32, kind="ExternalInput").ap()
    bq = nc.dram_tensor("bq", [D], F32, kind="ExternalInput").ap()
    bkx = nc.dram_tensor("bk_ext", [D + 1], F32, kind="ExternalInput").ap()
    out = nc.dram_tensor("out", [LQ, D], BF16, kind="ExternalOutput").ap()

    QB = LQ // 128  # 16 q blocks
    KB = LK // 128  # 16 k blocks
    IC1 = DIN // 128  # 8
    IC2 = D // 128  # 2

    with tile.TileContext(nc) as tc, ExitStack() as ctx:
        const = ctx.enter_context(tc.tile_pool(name="const", bufs=1))
        big = ctx.enter_context(tc.tile_pool(name="big", bufs=1))
        stage1 = ctx.enter_context(tc.tile_pool(name="stage1", bufs=2))
        stage2 = ctx.enter_context(tc.tile_pool(name="stage2", bufs=2))
        small = ctx.enter_context(tc.tile_pool(name="small", bufs=4))
        ps_gp = ctx.enter_context(tc.tile_pool(name="ps_gp", bufs=2, space="PSUM"))
        ps_sc = ctx.enter_context(tc.tile_pool(name="ps_sc", bufs=2, space="PSUM"))

        # ---------------- constants ----------------
        ones_row = const.tile([1, 128], BF16, tag="ones_row")
        nc.vector.memset(ones_row[:], 1.0)
        ident = const.tile([128, 128], F32, tag="ident")
        make_identity(nc, ident[:])
        ident_bf = const.tile([128, 128], BF16, tag="ident_bf")
        make_identity(nc, ident_bf[:])

        # ---------------- data loads first (sync queue), weights via SWDGE cast ----------------
        d2_st = [stage2.tile([128, 8 * D], F32, tag="d2st", name=f"d2st{g2}")
                 for g2 in range(2)]
        for g2 in range(2):
            for hh in range(2):
                nc.sync.dma_start(
                    out=d2_st[g2][:, hh * 4 * D:(hh + 1) * 4 * D].rearrange(
                        "p (t i) -> p t i", i=D),
                    in_=d2[g2 * 1024 + hh * 512: g2 * 1024 + (hh + 1) * 512, :]
                    .rearrange("(t p) i -> p t i", p=128),
                )
        d1_st = [stage1.tile([128, 4 * DIN], F32, tag="d1st", name=f"d1st{g}")
                 for g in range(4)]
        for g in range(2):
            nc.sync.dma_start(
                out=d1_st[g][:].rearrange("p (t i) -> p t i", i=DIN),
                in_=d1[g * 512:(g + 1) * 512, :].rearrange("(t p) i -> p t i", p=128),
            )
        bq_col = const.tile([128, IC2], F32, tag="bq_col")
        for c in range(IC2):
            nc.sync.dma_start(
                out=bq_col[:, c:c + 1],
                in_=bq[c * 128:(c + 1) * 128].rearrange("(p a) -> p a", a=1),
            )
        for g in range(2, 4):
            nc.sync.dma_start(
                out=d1_st[g][:].rearrange("p (t i) -> p t i", i=DIN),
                in_=d1[g * 512:(g + 1) * 512, :].rearrange("(t p) i -> p t i", p=128),
            )
        wq_sb = const.tile([128, IC1 * D], BF16, tag="wq_sb")
        nc.gpsimd.dma_start(
            out=wq_sb[:].rearrange("p (c d) -> p c d", d=D),
            in_=wq.rearrange("(c p) d -> p c d", p=128),
        )
        wk_sb = const.tile([128, IC2 * (D + 1)], BF16, tag="wk_sb")
        nc.gpsimd.dma_start(
            out=wk_sb[:].rearrange("p (c d) -> p c d", d=D + 1),
            in_=wkx.rearrange("(c p) d -> p c d", p=128),
        )
        wqs = [wq_sb[:, i * D:(i + 1) * D] for i in range(IC1)]
        wks = [wk_sb[:, i * (D + 1):(i + 1) * (D + 1)] for i in range(IC2)]
        bkx_row = const.tile([1, D + 1], BF16, tag="bkx_row")
        nc.gpsimd.dma_start(out=bkx_row[:], in_=bkx.rearrange("(a d) -> a d", a=1))

        # ---------------- data2: 2 group loads -> DVE cast -> PE transposes ----------------
        d2T = [big.tile([128, LK], BF16, tag=f"d2T{i}", name=f"d2T{i}") for i in range(IC2)]
        for g2 in range(2):  # 8 k-tiles per group
            st = d2_st[g2]
            bf = stage2.tile([128, 8 * D], BF16, tag="d2bf", name=f"d2bfs{g2}")
            for hh in range(2):
                nc.vector.tensor_copy(
                    bf[:, hh * 4 * D:(hh + 1) * 4 * D],
                    st[:, hh * 4 * D:(hh + 1) * 4 * D],
                )
            for ic in range(IC2):
                for h in range(2):  # 4 k-tiles per psum batch
                    pt = ps_gp.tile([128, 512], BF16, tag="ps_t",
                                    name=f"pt_d2_{g2}_{ic}_{h}")
                    for j in range(4):
                        kt = 4 * h + j
                        nc.tensor.transpose(
                            pt[:, j * 128:(j + 1) * 128],
                            bf[:, kt * D + ic * 128: kt * D + (ic + 1) * 128],
                            ident_bf[:],
                        )
                    nc.scalar.copy(
                        d2T[ic][:, g2 * 1024 + h * 512: g2 * 1024 + (h + 1) * 512],
                        pt[:],
                    )

        # ---------------- K^T fp8 DoubleRow layout [128, 2, k] (d = dc*128+p) ----------------
        kt_sb = big.tile([128, 2, LK], FP8, tag="kt_sb")
        for dc in range(2):
            for nk in range(LK // 512):
                ps = ps_gp.tile([128, 512], F32, tag="ps_gp")
                for ic in range(IC2):
                    nc.tensor.matmul(
                        ps[:],
                        lhsT=wks[ic][:, dc * 128:(dc + 1) * 128],
                        rhs=d2T[ic][:, nk * 512:(nk + 1) * 512],
                        start=(ic == 0),
                        stop=(ic == IC2 - 1),
                    )
                nc.vector.tensor_copy(kt_sb[:, dc, nk * 512:(nk + 1) * 512], ps[:])

        # ---------------- key value matrix fp8 pairs [128, 2, 257] = [key | 1] ----------------
        key2 = [
            big.tile([128, 2, D + 1], FP8, tag=f"key2_{kp}", name=f"key2_{kp}")
            for kp in range(KB // 2)
        ]
        def key_units():
            return [lambda kb=kb: key_unit(kb) for kb in range(KB)]

        def key_phase():
            for u in key_units():
                u()

        def key_unit(kb):
            if True:
                ps = ps_gp.tile([128, 512], F32, tag="ps_gp")
                p = ps[:, :D + 1]
                for ic in range(IC2):
                    nc.tensor.matmul(
                        p,
                        lhsT=d2T[ic][:, kb * 128:(kb + 1) * 128],
                        rhs=wks[ic],
                        start=(ic == 0),
                        stop=False,
                    )
                nc.tensor.matmul(p, lhsT=ones_row[:], rhs=bkx_row[:], start=False, stop=True)
                nc.vector.tensor_copy(key2[kb // 2][:, kb % 2, :], p)

        # ---------------- data1: 4 group loads -> DVE cast -> PE transposes ----------------
        d1T = [big.tile([128, LQ], BF16, tag=f"d1T{i}", name=f"d1T{i}") for i in range(IC1)]

        def d1_group(g):
            st = d1_st[g]
            bf = stage1.tile([128, 4 * DIN], BF16, tag="d1bf", name=f"d1bfs{g}", bufs=2)
            nc.vector.tensor_copy(bf[:], st[:])
            for ic in range(IC1):
                pt = ps_gp.tile([128, 512], BF16, tag="ps_t", name=f"pt_d1_{g}_{ic}")
                for j in range(4):
                    nc.tensor.transpose(
                        pt[:, j * 128:(j + 1) * 128],
                        bf[:, j * DIN + ic * 128: j * DIN + (ic + 1) * 128],
                        ident_bf[:],
                    )
                nc.vector.tensor_copy(d1T[ic][:, g * 512:(g + 1) * 512], pt[:])

        # only the first half of data1 before QT-h0; groups 2-3 are emitted
        # after the scores-h0 interleave so load waits never block the PE
        # queue ahead of the exp-feeding scores matmuls
        d1_group(0)
        d1_group(1)

        # ---------------- per-half pipeline ----------------
        q_sb = [big.tile([128, D], BF16, tag=f"q{qb}", name=f"q{qb}") for qb in range(QB)]
        qt_sb = big.tile([128, 2, LQ], FP8, tag="qt_sb")
        qtbf = big.tile([128, 2, LQ], BF16, tag="qtbf")
        expT = [
            [big.tile([128, 2, 1024], FP8, tag=f"expT{kp}_{nh}", name=f"expT{kp}_{nh}")
             for nh in range(2)]
            for kp in range(KB // 2)
        ]
        out_sb = [
            big.tile([128, 4 * D], BF16, tag=f"out_sb{hq}", name=f"out_sb{hq}")
            for hq in range(4)
        ]

        def qt_units(h):
            units = []
            for dc in range(2):
                for nq in range(h * 2, h * 2 + 2):
                    units.append(lambda dc=dc, nq=nq: qt_unit(dc, nq))
            return units

        def qt_phase(h):
            for u in qt_units(h):
                u()

        def qt_unit(dc, nq):
            if True:
                if True:
                    ps = ps_gp.tile([128, 512], F32, tag="ps_gp")
                    for ic in range(IC1):
                        nc.tensor.matmul(
                            ps[:],
                            lhsT=wqs[ic][:, dc * 128:(dc + 1) * 128],
                            rhs=d1T[ic][:, nq * 512:(nq + 1) * 512],
                            start=(ic == 0),
                            stop=(ic == IC1 - 1),
                        )
                    if nq < 2:
                        # pre-scores window: ACT is idle, use it
                        nc.scalar.activation(
                            qt_sb[:, dc, nq * 512:(nq + 1) * 512], ps[:], AF.Identity,
                            bias=bq_col[:, dc:dc + 1],
                        )
                        nc.scalar.activation(
                            qtbf[:, dc, nq * 512:(nq + 1) * 512], ps[:], AF.Identity,
                            bias=bq_col[:, dc:dc + 1],
                        )
                    else:
                        # interleaved with scores exps: keep off the in-order
                        # ACT queue (PSUM-wait would head-of-line-block exp)
                        nc.vector.tensor_scalar(
                            qt_sb[:, dc, nq * 512:(nq + 1) * 512], ps[:],
                            bq_col[:, dc:dc + 1], None, mybir.AluOpType.add,
                        )
                        nc.vector.tensor_scalar(
                            qtbf[:, dc, nq * 512:(nq + 1) * 512], ps[:],
                            bq_col[:, dc:dc + 1], None, mybir.AluOpType.add,
                        )

        def qres_units(h):
            units = []
            for qg in range(h * 2, h * 2 + 2):
                for dc in range(2):
                    units.append(lambda qg=qg, dc=dc: qres_unit(qg, dc))
            return units

        def qres_phase(h):
            for u in qres_units(h):
                u()

        def qres_unit(qg, dc):
            if True:
                if True:
                    pt = ps_gp.tile([128, 512], BF16, tag="ps_t",
                                    name=f"pt_q_{qg}_{dc}")
                    for j in range(4):
                        qb = qg * 4 + j
                        nc.tensor.transpose(
                            pt[:, j * 128:(j + 1) * 128],
                            qtbf[:, dc, qb * 128:(qb + 1) * 128],
                            ident_bf[:],
                        )
                    for j in range(4):
                        qb = qg * 4 + j
                        nc.vector.tensor_copy(
                            q_sb[qb][:, dc * 128:(dc + 1) * 128],
                            pt[:, j * 128:(j + 1) * 128],
                        )

        def scores_units(nh):
            return [lambda km=km: scores_unit(km, nh) for km in range(KB)]

        def scores_unit(km, nh):
            if True:
                ps = ps_sc.tile([128, 1024], F32, tag="ps_sc")
                for half in range(2):
                    nq = nh * 2 + half
                    nc.tensor.matmul(
                        ps[:, half * 512:(half + 1) * 512],
                        lhsT=kt_sb[:, :, km * 128:(km + 1) * 128],
                        rhs=qt_sb[:, :, nq * 512:(nq + 1) * 512],
                        perf_mode=PM_DR,
                        start=True,
                        stop=True,
                    )
                nc.scalar.activation(
                    expT[km // 2][nh][:, km % 2, :], ps[:], AF.Exp, scale=SCALE
                )

        def ctx_units(h):
            return [lambda qb=qb: ctx_unit(qb) for qb in range(h * 8, h * 8 + 8)]

        def ctx_phase(h):
            for u in ctx_units(h):
                u()

        def ctx_unit(qb):
            if True:
                h, qq = qb // 8, qb % 8
                hq, qqq = qb // 4, qb % 4
                if h == 0:
                    pc_full = ps_gp.tile([128, 512], F32, tag="ps_gp")
                else:
                    # scores pool is idle once scores-h1 is done; borrow it so
                    # the tail context chains don't contend with qres psum use
                    pc_full = ps_sc.tile([128, 512], F32, tag="ps_sc")
                pc = pc_full[:, :D + 1]
                for kp in range(KB // 2):
                    nc.tensor.matmul(
                        pc,
                        lhsT=expT[kp][h][:, :, qq * 128:(qq + 1) * 128],
                        rhs=key2[kp][:],
                        perf_mode=PM_DR,
                        start=(kp == 0),
                        stop=(kp == KB // 2 - 1),
                    )
                rc = small.tile([128, 1], F32, tag="recip")
                nc.vector.reciprocal(rc[:], pc[:, D:D + 1])
                osl = out_sb[hq][:, qqq * D:(qqq + 1) * D]
                nc.vector.tensor_scalar(osl, pc[:, :D], rc[:], None,
                                        mybir.AluOpType.mult)
                nc.vector.tensor_add(osl, osl, q_sb[qb][:])
                if qqq == 3:
                    nc.sync.dma_start(
                        out=out[hq * (LQ // 4):(hq + 1) * (LQ // 4), :].rearrange(
                            "(qt p) d -> p qt d", p=128
                        ),
                        in_=out_sb[hq][:].rearrange("p (qt d) -> p qt d", d=D),
                    )

        def interleave(a, b, ratio):
            a = list(a); b = list(b)
            ia = ib = 0
            while ia < len(a) or ib < len(b):
                for _ in range(ratio):
                    if ia < len(a):
                        a[ia](); ia += 1
                if ib < len(b):
                    b[ib](); ib += 1

        qt_phase(0)
        # scores-h0 interleaved ONLY with early-ready filler (key + qres-h0);
        # anything waiting on late data1 loads would head-of-line block the
        # PE queue ahead of the exp-feeding scores matmuls
        interleave(scores_units(0), key_units() + qres_units(0), 1)
        d1_group(2)
        d1_group(3)
        qt_phase(1)
        # scores-h1 interleaved with ctx-h0
        interleave(scores_units(1), ctx_units(0), 2)
        qr1 = qres_units(1)
        cx1 = ctx_units(1)
        qr1[0](); qr1[1]()
        cx1[0](); cx1[1]()
        qr1[2](); qr1[3]()
        for u in cx1[2:]:
            u()

    nc.compile()
    return nc


_NC = None


def _get_nc():
    global _NC
    if _NC is None:
        _NC = _build()
    return _NC


def kernel(data1, data2, Wq, bq, Wk, bk):
    data1 = np.asarray(data1, dtype=np.float32)
    data2 = np.asarray(data2, dtype=np.float32)
    Wq = np.ascontiguousarray(np.asarray(Wq, dtype=np.float32))
    bq = np.ascontiguousarray(np.asarray(bq, dtype=np.float32))
    Wk = np.asarray(Wk, dtype=np.float32)
    bk = np.asarray(bk, dtype=np.float32)

    wk_ext = np.zeros((D, D + 1), dtype=np.float32)
    wk_ext[:, :D] = Wk
    bk_ext = np.concatenate([bk, np.ones(1, dtype=np.float32)]).astype(np.float32)

    nc = _get_nc()
    in_maps = [
        {
            "data1": np.ascontiguousarray(data1[b]),
            "data2": np.ascontiguousarray(data2[b]),
            "Wq": Wq,
            "Wk_ext": wk_ext,
            "bq": bq,
            "bk_ext": bk_ext,
        }
        for b in range(B)
    ]
    res = run_bass_kernel_spmd(nc, in_maps, core_ids=list(range(N_CORES)))
    return np.stack(
        [np.asarray(res.results[i]["out"]).astype(np.float32) for i in range(B)],
        axis=0,
    )

